# revision 20
# baseline (speedup 1.0000x reference)
"""CTC loss kernel for Trainium2 (8 NeuronCores, data-parallel over batch).

Strategy
--------
Per core: 64 examples. The CTC forward DP runs in probability space
with states in the free dim and (example, direction) packed into the
128 partitions: rows 0-63 run the forward DP for t=0..255, rows 64-127
run the suffix (backward) DP in state-reversed coordinates for
t=511..256.  The two halves are spliced at T/2 on host in f64:
P = sum_s alpha_255[s] * W_255[s].

Emissions E[b,t,s] = g*y_pred[b,t,ext_b[s]] are produced on the
TensorEngine: per (example, 128-t block), PE-transpose y_pred to
(class, t), then a one-hot matmul gathers all 132 state emissions for
128 t steps in one instruction; the scalar engine copies PSUM->SBUF
fusing the g scale and the f32->bf16 cast.  A DRAM round-trip
reshuffles (t-part, ex, s) into (example-part, tau-major) 32-step
chunks, landing in the ec half of an interleaved [ecm|ec] layout; a
bulk DVE multiply by the replicated skip mask fills ecm = m2*ec.

The DP is split into two independent interleaved strands so every DVE
dependency is >= 2 instructions back and same-engine semaphore waits
resolve for free (a naive schedule pays ~91ns per distance-1 wait,
3x per step): strand A owns mirrored states [0:66) plus a 16-col
ghost, strand B owns [66:132).  Mirrored state order (guards at top)
makes information flow upward-only, so B needs no ghost; A's ghost
absorbs the 2-col/step creep and is refreshed from B every 8 steps.
Each strand step is 3 ops on a packed tile T = [alpha | guard(2) | u]:
  u = alpha[0:w]+alpha[1:w+1];  W = T[2:2+2w]*[ecm|ec](strided 2-row);
  alpha' = W[0:w]+W[w:2w]
with all operands 4-byte aligned to hit the DVE 2x bf16 perf mode.

Numerics: bf16 DP state, per-32-step rescale to a 2^24 setpoint via
the fast approx reciprocal (max history written out, logs added back
on host); masked/pad one-hot columns are exactly zero (dropping the
log(y+eps) eps shifts the loss ~1e-5 relative, far under tolerance).
The final splice spans ~e^-180 for tail examples, far outside f32
range, so the final state tiles are DMA'd out and spliced on host.
"""

import numpy as np

B, T, C, L = 512, 512, 96, 64
BLANK = C - 1
EPS = 1e-7
S = 2 * L + 1          # 129 states
SW = 132               # padded state width (multiple of 4)
NCORES = 8
BN = B // NCORES       # 64 examples per core
TH = T // 2            # split point
RESC = 32              # rescale period
NRESC = (TH - 1) // RESC  # 7 rescales (tau = 32,64,...,224)
SETPOINT_LOG2 = 24     # rescale setpoint 2^24 (headroom for 32 unrescaled steps)
G = 60.646622          # exp(mean_loss/T) boost; keeps alpha ~O(1) per step
EW = 2 * SW            # per-tau DP emission width: [ecm | ec]
SA = 66                # strand A owns mirrored states [0:SA)
GREF = 8               # ghost refresh period
GH = 2 * GREF          # ghost width (16)
WA = SA + GH           # strand A computed width (82)
WB = SW - SA           # strand B width (66)

CHK_ = 32
_BUILT = None
_LAST_EXEC_NS = None
_LAST_RES = None


def _host_metadata(y_true):
    """ext labels, skip masks, init masks, per-state classes — from y_true.

    Everything is built in natural state order (validated layout), then
    reversed along the free dim at pack time for the mirrored device layout.
    """
    y_true = np.asarray(y_true, dtype=np.int32)
    lbl_len = (y_true != -1).sum(axis=-1).astype(np.int32)
    labels = np.where(y_true != -1, y_true, 0).astype(np.int32)
    ext = np.full((B, S), BLANK, np.int32)
    ext[:, 1::2] = labels
    ext_m2 = np.pad(ext[:, :-2], ((0, 0), (2, 0)), constant_values=BLANK)
    can_skip = ((ext != BLANK) & (ext != ext_m2)).astype(np.float32)

    m2f = np.zeros((B, SW), np.float32)
    m2f[:, :S] = can_skip
    m2b = np.zeros((B, SW), np.float32)
    for u in range(2, S):
        m2b[:, u] = can_skip[:, S - 1 - u + 2]

    mif = np.zeros((B, SW), np.float32)
    mif[:, 0] = 1.0
    mif[:, 1] = 1.0
    mib = np.zeros((B, SW), np.float32)
    mib[np.arange(B), S - 1 - 2 * lbl_len] = 1.0
    mib[np.arange(B), S - 1 - (2 * lbl_len - 1)] = 1.0

    clsf = np.full((B, SW), -1, np.int32)           # -1 -> all-zero column
    clsf[:, :S] = ext
    clsb = np.full((B, SW), -1, np.int32)
    clsb[:, :S] = ext[:, ::-1]
    return m2f, m2b, mif, mib, clsf, clsb


def _build(num_cores=NCORES, t_full=T, bn=BN):
    """Build and schedule the Bass module once."""
    import concourse.bacc as bacc
    import concourse.mybir as mybir
    import concourse.tile as tile
    from contextlib import ExitStack
    from concourse.vector_clock import ScopedClock

    # this walrus build allows a single sem wait per Drain: split the
    # TileContext end-drain's waits across a chain of drains.
    def _patched_drain_and_barrier(self, tick_clock, wait_clock):
        nc = self.nc
        drain_inst = nc.sync.drain()
        wait_clock.add_sem_waits(
            drain_inst.ins, ScopedClock({None: tick_clock.global_clock})
        )
        si = drain_inst.ins.sync_info
        waits = list(si.on_wait) if si and si.on_wait else []
        if len(waits) > 1:
            si.on_wait = waits[:1]
            for w in waits[1:]:
                extra = nc.sync.drain()
                esi = extra.ins.sync_info
                if esi is None:
                    extra.ins.sync_info = mybir.SyncInfo(on_wait=[w], on_update=[])
                else:
                    esi.on_wait = (esi.on_wait or []) + [w]
        nc.all_engine_barrier()
        assert self.sems is not None
        popped = nc._tile_sem_poison_stack.pop()
        assert popped is self._sem_poison
        nc.clear_and_free_semaphores(list(self.sems.allocated().values()))
        nc.all_engine_barrier()

    tile.TileContext._drain_and_barrier = _patched_drain_and_barrier

    f32 = mybir.dt.float32
    bf16 = mybir.dt.bfloat16
    AX = mybir.AxisListType.X
    COPY = mybir.ActivationFunctionType.Copy
    MULT = mybir.AluOpType.mult

    th = t_full // 2
    nblk = t_full // 128
    chk = CHK_
    nchk = th // chk
    nresc = (th - 1) // RESC

    nc = bacc.Bacc("TRN2", target_bir_lowering=False, debug=False,
                   num_devices=num_cores)
    # block-major, class-padded bf16 y_pred: [blk][ex][t-in-blk][c-pad-128]
    ypred = nc.dram_tensor("ypredb", [nblk, bn, 128, 128], bf16,
                           kind="ExternalInput")
    m2_in = nc.dram_tensor("m2", [128, SW], bf16, kind="ExternalInput")
    m2r_in = nc.dram_tensor("m2rep", [128, CHK_ * SW], bf16, kind="ExternalInput")
    mi_in = nc.dram_tensor("minit", [128, SW], bf16, kind="ExternalInput")
    oh_in = nc.dram_tensor("onehot", [C, bn * 2 * SW], bf16, kind="ExternalInput")
    h_out = nc.dram_tensor("hist", [128, max(nresc, 1)], f32, kind="ExternalOutput")
    a_out = nc.dram_tensor("afin", [128, SW + 2], bf16, kind="ExternalOutput")
    g_out = nc.dram_tensor("gfin", [128, SW], bf16, kind="ExternalOutput")

    with tile.TileContext(nc) as tc, ExitStack() as ctx:
        const = ctx.enter_context(tc.tile_pool(name="const", bufs=1))
        dramp = ctx.enter_context(tc.tile_pool(name="edram", bufs=1, space="DRAM"))
        ebf_pool = ctx.enter_context(tc.tile_pool(name="ebf", bufs=3))
        ec_pool = ctx.enter_context(tc.tile_pool(name="ec", bufs=4))
        eps_pool = ctx.enter_context(tc.tile_pool(name="eps", bufs=6, space="PSUM"))
        yts_pool = ctx.enter_context(tc.tile_pool(name="yts", bufs=2))

        m2t = const.tile([128, SW], bf16, tag="m2t")
        m2rep = const.tile([128, CHK_ * SW], bf16, tag="m2rep")
        TA = const.tile([128, 2 * WA + 2], bf16, tag="TA")
        TB = const.tile([128, 2 * WB + 2], bf16, tag="TB")
        WAt = const.tile([128, 2 * WA], bf16, tag="WAt")
        WBt = const.tile([128, 2 * WB], bf16, tag="WBt")
        alf = const.tile([128, SW + 2], bf16, tag="alf")
        hB = const.tile([128, 1], f32, tag="hB")
        mit = const.tile([128, SW], bf16, tag="mit")
        oht = const.tile([C, bn * 2 * SW], bf16, tag="oht")
        alpha = const.tile([128, SW + 2], bf16, tag="alpha")
        ut = const.tile([128, SW], bf16, tag="ut")
        vt = const.tile([128, SW], bf16, tag="vt")
        wt = const.tile([128, SW], bf16, tag="wt")
        histt = const.tile([128, max(nresc, 1)], f32, tag="histt")
        sclt = const.tile([128, 1], f32, tag="sclt")

        blk_order = []
        for i in range(nblk // 2):
            blk_order += [i, nblk - 1 - i]
        pair1, pair2 = blk_order[:2], blk_order[2:]

        nc.sync.dma_start(out=m2t[:, :], in_=m2_in.ap())
        nc.sync.dma_start(out=m2rep[:, :], in_=m2r_in.ap())
        nc.sync.dma_start(out=mit[:, :], in_=mi_in.ap())
        gw = 16 * 2 * SW

        def load_oh(g):
            nc.sync.dma_start(out=oht[:, g * gw:(g + 1) * gw],
                              in_=oh_in.ap()[:, g * gw:(g + 1) * gw])

        load_oh(0)
        load_oh(1)
        nc.vector.memset(histt[:, :], 0.0)
        nc.vector.memset(TA[:, :], 0.0)
        nc.vector.memset(TB[:, :], 0.0)
        nc.vector.memset(alf[:, :], 0.0)

        # ---- phase A helper: emissions per 128-t block via one-hot matmul ----
        edram = {}

        def emit_block(k, grp_hook=None):
            ebf_k = ebf_pool.tile([128, bn * SW], bf16)
            dirn = 0 if k < nblk // 2 else 1
            ed = dramp.tile([128, bn * SW], bf16, tag=f"ed{k}")
            edram[k] = ed
            # one yts tile per block: [c(128), ex*128t] via XBAR DMA
            # transpose straight from block-major DRAM (16-example slabs)
            ysb = yts_pool.tile([128, bn * 128], bf16)
            for grp in range(bn // 16):
                src = ypred.ap()[k, grp * 16:(grp + 1) * 16, :, :]
                nc.sync.dma_start(
                    out=ysb[:, grp * 16 * 128:(grp + 1) * 16 * 128],
                    in_=src.rearrange("e t c -> (e t) c"), transpose=True)
            for grp in range(bn // 16):
                if grp_hook is not None:
                    grp_hook(grp)
                for e in range(16):
                    ex = grp * 16 + e
                    ohoff = (ex * 2 + dirn) * SW
                    epsum = eps_pool.tile([128, SW], f32)
                    nc.tensor.matmul(
                        epsum[:, :],
                        ysb[0:C, ex * 128:(ex + 1) * 128],
                        oht[:, ohoff:ohoff + SW],
                        start=True, stop=True)
                    nc.scalar.activation(
                        ebf_k[:, ex * SW:(ex + 1) * SW], epsum[:, :],
                        COPY, bias=float(G * EPS), scale=float(G))
                # per-grp DRAM write: the last grp's write is all that
                # gates the chunk reads, not the whole block
                gsl = slice(grp * 16 * SW, (grp + 1) * 16 * SW)
                nc.gpsimd.dma_start(out=ed[:, gsl], in_=ebf_k[:, gsl])

        # ---- phase B helper: reshuffle DRAM -> (ex|dir partition, tau) ----
        ec_tiles = []
        m2rep3 = m2rep[:, :].rearrange("p (t s) -> p t s", s=SW)

        def emit_chunk_fwd(j):
            ec = ec_pool.tile([128, chk * EW], bf16)
            ec3 = ec[:, :].rearrange("p (t s) -> p t s", s=EW)
            kf = j // 4
            tl0 = (j % 4) * chk
            fsrc = edram[kf][:, :].rearrange("t (e s) -> t e s", s=SW)
            # Pool queue: ordered after the ed writes it depends on; cheap
            # SWDGE dispatch keeps the SP queue free for XBAR transposes
            nc.gpsimd.dma_start(
                out=ec3[0:64, :, SW:EW],
                in_=fsrc[tl0:tl0 + chk, :, :].rearrange("t e s -> e t s"))
            ec_tiles.append((ec, ec3))

        def emit_chunk_bwd(j):
            ec, ec3 = ec_tiles[j]
            kb = nblk - 1 - j // 4
            tb0 = 127 - (j % 4) * chk
            bsrc = edram[kb][:, :].rearrange("t (e s) -> t e s", s=SW)
            bslice = slice(tb0, None, -1) if tb0 - chk < 0 else slice(tb0, tb0 - chk, -1)
            nc.gpsimd.dma_start(
                out=ec3[64:128, :, SW:EW],
                in_=bsrc[bslice, :, :].rearrange("t e s -> e t s"))

        def emit_chunk_mul(j):
            ec, ec3 = ec_tiles[j]
            nc.vector.tensor_mul(ec3[:, :, 0:SW], ec3[:, :, SW:EW], m2rep3)

        # ---- phase C: two-strand interleaved DP (deps all >= dist 2) ----
        MAXO = mybir.AluOpType.max
        nr_box = [0]

        def emit_dp(tau0, tau1):
            for tau in range(tau0, tau1):
                if tau % chk == 0:
                    emit_chunk_mul(tau // chk)
                ecc = ec_tiles[tau // chk][0]
                off = (tau % chk) * EW
                ecv = ecc[:, off:off + EW].rearrange("p (h s) -> p h s", h=2)
                nc.vector.tensor_add(TA[:, WA + 2:2 * WA + 2],
                                     TA[:, 0:WA], TA[:, 1:1 + WA])
                nc.vector.tensor_add(TB[:, WB + 2:2 * WB + 2],
                                     TB[:, 0:WB], TB[:, 1:1 + WB])
                nc.vector.tensor_mul(
                    WAt[:, :].rearrange("p (h s) -> p h s", h=2),
                    TA[:, 2:2 * WA + 2].rearrange("p (h s) -> p h s", h=2),
                    ecv[:, :, 0:WA])
                nc.vector.tensor_mul(
                    WBt[:, :].rearrange("p (h s) -> p h s", h=2),
                    TB[:, 2:2 * WB + 2].rearrange("p (h s) -> p h s", h=2),
                    ecv[:, :, SA:SW])
                nc.vector.tensor_add(TA[:, 0:WA], WAt[:, 0:WA], WAt[:, WA:2 * WA])
                nc.vector.tensor_add(TB[:, 0:WB], WBt[:, 0:WB], WBt[:, WB:2 * WB])
                if tau % GREF == 0:
                    # refresh A's ghost cols from B (unscaled; rescale covers both)
                    nc.vector.tensor_copy(TA[:, SA:WA], TB[:, 0:GH])
                if tau % RESC == 0 and nr_box[0] < nresc:
                    nr = nr_box[0]
                    nc.vector.reduce_max(histt[:, nr:nr + 1], TA[:, 2:SA], axis=AX)
                    nc.vector.reduce_max(hB[:, :], TB[:, 0:WB], axis=AX)
                    nc.vector.tensor_max(histt[:, nr:nr + 1],
                                         histt[:, nr:nr + 1], hB[:, :])
                    nc.vector.reciprocal_approx_fast(sclt[:, :], histt[:, nr:nr + 1])
                    nc.vector.tensor_scalar(TB[:, 0:WB], TB[:, 0:WB],
                                            sclt[:, :], float(2.0 ** SETPOINT_LOG2),
                                            MULT, MULT)
                    nc.vector.tensor_scalar(TA[:, 0:WA], TA[:, 0:WA],
                                            sclt[:, :], float(2.0 ** SETPOINT_LOG2),
                                            MULT, MULT)
                    nr_box[0] = nr + 1

        # ---- interleaved emission: DP starts once pair-1 chunks land ----
        def _oh_hook(g):
            if g < 2:
                load_oh(g + 2)

        emit_block(pair1[0], grp_hook=_oh_hook)
        emit_block(pair1[1])
        for j in range(4):
            emit_chunk_fwd(j)
        for j in range(4):
            emit_chunk_bwd(j)

        emit_chunk_mul(0)
        ecc0 = ec_tiles[0][0]
        nc.vector.tensor_mul(TA[:, 0:WA], ecc0[:, SW:SW + WA], mit[:, 0:WA])
        nc.vector.tensor_mul(TB[:, 0:WB], ecc0[:, SW + SA:EW], mit[:, SA:SW])
        emit_dp(1, 4 * chk)

        emit_block(pair2[0])
        emit_block(pair2[1])
        for j in range(4, nchk):
            emit_chunk_fwd(j)
        for j in range(4, nchk):
            emit_chunk_bwd(j)
        emit_dp(4 * chk, th)

        # ---- final: assemble alpha, gamma on bwd rows, dump ----
        nc.vector.tensor_copy(alf[:, 0:SA], TA[:, 0:SA])
        nc.vector.tensor_copy(alf[:, SA:SW], TB[:, 0:WB])
        nc.vector.tensor_add(ut[:, :], alf[:, 0:SW], alf[:, 1:1 + SW])
        nc.vector.tensor_mul(vt[:, :], alf[:, 2:2 + SW], m2t[:, :])
        nc.vector.tensor_add(wt[:, :], ut[:, :], vt[:, :])
        nc.sync.dma_start(out=a_out.ap(), in_=alf[:, :])
        nc.sync.dma_start(out=g_out.ap(), in_=wt[:, :])
        nc.sync.dma_start(out=h_out.ap(), in_=histt[:, :])

    nc.compile()
    return nc


def kernel(y_true, y_pred):
    global _BUILT, _LAST_EXEC_NS, _LAST_RES
    from concourse.bass_utils import run_bass_kernel_spmd

    y_true = np.asarray(y_true)
    y_pred = np.ascontiguousarray(np.asarray(y_pred, dtype=np.float32))

    m2f, m2b, mif, mib, clsf, clsb = _host_metadata(y_true)

    if _BUILT is None:
        _BUILT = _build()
    nc = _BUILT

    import ml_dtypes
    bf = ml_dtypes.bfloat16
    # block-major, class-padded bf16 y_pred: [4][512 ex][128 t][128 c]
    nblk = T // 128
    ypb_all = np.zeros((B, nblk, 128, 128), bf)
    ypb_all[:, :, :, :C] = y_pred.reshape(B, nblk, 128, C).astype(bf)
    ypb_all = np.ascontiguousarray(np.swapaxes(ypb_all, 0, 1))  # [blk, B, t, c]
    in_maps = []
    for c in range(NCORES):
        sl = slice(c * BN, (c + 1) * BN)
        # mirrored layout: reverse the free (state) dim
        m2 = np.concatenate([m2f[sl], m2b[sl]], axis=0)[:, ::-1].astype(bf)
        mi = np.concatenate([mif[sl], mib[sl]], axis=0)[:, ::-1].astype(bf)
        m2rep = np.tile(m2, (1, CHK_))
        oh = np.zeros((C, BN * 2 * SW), bf)
        for e in range(BN):
            b = c * BN + e
            for dirn, cls in ((0, clsf[b]), (1, clsb[b])):
                colbase = (e * 2 + dirn) * SW
                rcls = cls[::-1]                    # mirrored
                for i in range(SW):
                    if rcls[i] >= 0:
                        oh[rcls[i], colbase + i] = bf(1.0)
        in_maps.append({
            "ypredb": np.ascontiguousarray(ypb_all[:, sl]),
            "m2": np.ascontiguousarray(m2),
            "m2rep": np.ascontiguousarray(m2rep),
            "minit": np.ascontiguousarray(mi),
            "onehot": oh,
        })

    import os
    trace = os.environ.get("CTC_TRACE", "") == "1"
    res = run_bass_kernel_spmd(nc, in_maps, list(range(NCORES)), trace=trace)
    _LAST_EXEC_NS = res.exec_time_ns
    _LAST_RES = res

    losses = np.zeros(B, np.float64)
    lng = np.log(np.float64(G))
    setlog = NRESC * SETPOINT_LOG2 * np.log(2.0)
    for c in range(NCORES):
        afin = res.results[c]["afin"].astype(np.float64)   # (128, SW+2) mirrored
        gfin = res.results[c]["gfin"].astype(np.float64)   # (128, SW) mirrored
        hist = res.results[c]["hist"].astype(np.float64)
        acc = np.log(np.maximum(hist[:, :NRESC], 1e-300)).sum(axis=1)
        afs = afin[:, 0:SW][:, ::-1]             # un-mirror -> natural state order
        gfs = gfin[:, :][:, ::-1]
        af = afs[0:64, 0:S]                      # alpha_{T/2-1}[s]
        gm = gfs[64:128, 0:S][:, ::-1]           # W_{T/2-1}[s], u -> s
        P = (af * gm).sum(axis=1)
        lnP = np.log(np.maximum(P, 1e-300))
        losses[c * BN:(c + 1) * BN] = -(
            lnP + acc[:64] + acc[64:128] - 2 * setlog - T * lng)
    return np.float32(losses.mean())



# revision 21
# speedup vs baseline: 1.1053x; 1.1053x over previous
"""CTC loss kernel for Trainium2 (8 NeuronCores, data-parallel over batch).

Strategy
--------
Per core: 64 examples. The CTC forward DP runs in probability space
with states in the free dim and (example, direction) packed into the
128 partitions: rows 0-63 run the forward DP for t=0..255, rows 64-127
run the suffix (backward) DP in state-reversed coordinates for
t=511..256.  The two halves are spliced at T/2 on host in f64:
P = sum_s alpha_255[s] * W_255[s].

Emissions E[b,t,s] = g*y_pred[b,t,ext_b[s]] are produced on the
TensorEngine: per (example, 128-t block), PE-transpose y_pred to
(class, t), then a one-hot matmul gathers all 132 state emissions for
128 t steps in one instruction; the scalar engine copies PSUM->SBUF
fusing the g scale and the f32->bf16 cast.  A DRAM round-trip
reshuffles (t-part, ex, s) into (example-part, tau-major) 32-step
chunks, landing in the ec half of an interleaved [ecm|ec] layout; a
bulk DVE multiply by the replicated skip mask fills ecm = m2*ec.

The DP is split into two independent interleaved strands so every DVE
dependency is >= 2 instructions back and same-engine semaphore waits
resolve for free (a naive schedule pays ~91ns per distance-1 wait,
3x per step): strand A owns mirrored states [0:66) plus a 16-col
ghost, strand B owns [66:132).  Mirrored state order (guards at top)
makes information flow upward-only, so B needs no ghost; A's ghost
absorbs the 2-col/step creep and is refreshed from B every 8 steps.
Each strand step is 3 ops on a packed tile T = [alpha | guard(2) | u]:
  u = alpha[0:w]+alpha[1:w+1];  W = T[2:2+2w]*[ecm|ec](strided 2-row);
  alpha' = W[0:w]+W[w:2w]
with all operands 4-byte aligned to hit the DVE 2x bf16 perf mode.

Numerics: bf16 DP state, per-32-step rescale to a 2^24 setpoint via
the fast approx reciprocal (max history written out, logs added back
on host); masked/pad one-hot columns are exactly zero (dropping the
log(y+eps) eps shifts the loss ~1e-5 relative, far under tolerance).
The final splice spans ~e^-180 for tail examples, far outside f32
range, so the final state tiles are DMA'd out and spliced on host.
"""

import numpy as np

B, T, C, L = 512, 512, 96, 64
BLANK = C - 1
EPS = 1e-7
S = 2 * L + 1          # 129 states
SW = 132               # padded state width (multiple of 4)
NCORES = 8
BN = B // NCORES       # 64 examples per core
TH = T // 2            # split point
RESC = 32              # rescale period
NRESC = (TH - 1) // RESC  # 7 rescales (tau = 32,64,...,224)
SETPOINT_LOG2 = 24     # rescale setpoint 2^24 (headroom for 32 unrescaled steps)
G = 60.646622          # exp(mean_loss/T) boost; keeps alpha ~O(1) per step
EW = 2 * SW            # per-tau DP emission width: [ecm | ec]
SA = 66                # strand A owns mirrored states [0:SA)
GREF = 8               # ghost refresh period
GH = 2 * GREF          # ghost width (16)
WA = SA + GH           # strand A computed width (82)
WB = SW - SA           # strand B width (66)

CHK_ = 32
_BUILT = None
_LAST_EXEC_NS = None
_LAST_RES = None


def _host_metadata(y_true):
    """ext labels, skip masks, init masks, per-state classes — from y_true.

    Everything is built in natural state order (validated layout), then
    reversed along the free dim at pack time for the mirrored device layout.
    """
    y_true = np.asarray(y_true, dtype=np.int32)
    lbl_len = (y_true != -1).sum(axis=-1).astype(np.int32)
    labels = np.where(y_true != -1, y_true, 0).astype(np.int32)
    ext = np.full((B, S), BLANK, np.int32)
    ext[:, 1::2] = labels
    ext_m2 = np.pad(ext[:, :-2], ((0, 0), (2, 0)), constant_values=BLANK)
    can_skip = ((ext != BLANK) & (ext != ext_m2)).astype(np.float32)

    m2f = np.zeros((B, SW), np.float32)
    m2f[:, :S] = can_skip
    m2b = np.zeros((B, SW), np.float32)
    for u in range(2, S):
        m2b[:, u] = can_skip[:, S - 1 - u + 2]

    mif = np.zeros((B, SW), np.float32)
    mif[:, 0] = 1.0
    mif[:, 1] = 1.0
    mib = np.zeros((B, SW), np.float32)
    mib[np.arange(B), S - 1 - 2 * lbl_len] = 1.0
    mib[np.arange(B), S - 1 - (2 * lbl_len - 1)] = 1.0

    clsf = np.full((B, SW), -1, np.int32)           # -1 -> all-zero column
    clsf[:, :S] = ext
    clsb = np.full((B, SW), -1, np.int32)
    clsb[:, :S] = ext[:, ::-1]
    return m2f, m2b, mif, mib, clsf, clsb


def _build(num_cores=NCORES, t_full=T, bn=BN):
    """Build and schedule the Bass module once."""
    import concourse.bacc as bacc
    import concourse.mybir as mybir
    import concourse.tile as tile
    from contextlib import ExitStack
    from concourse.vector_clock import ScopedClock

    # this walrus build allows a single sem wait per Drain: split the
    # TileContext end-drain's waits across a chain of drains.
    def _patched_drain_and_barrier(self, tick_clock, wait_clock):
        nc = self.nc
        drain_inst = nc.sync.drain()
        wait_clock.add_sem_waits(
            drain_inst.ins, ScopedClock({None: tick_clock.global_clock})
        )
        si = drain_inst.ins.sync_info
        waits = list(si.on_wait) if si and si.on_wait else []
        if len(waits) > 1:
            si.on_wait = waits[:1]
            for w in waits[1:]:
                extra = nc.sync.drain()
                esi = extra.ins.sync_info
                if esi is None:
                    extra.ins.sync_info = mybir.SyncInfo(on_wait=[w], on_update=[])
                else:
                    esi.on_wait = (esi.on_wait or []) + [w]
        nc.all_engine_barrier()
        assert self.sems is not None
        popped = nc._tile_sem_poison_stack.pop()
        assert popped is self._sem_poison
        nc.clear_and_free_semaphores(list(self.sems.allocated().values()))
        nc.all_engine_barrier()

    tile.TileContext._drain_and_barrier = _patched_drain_and_barrier

    f32 = mybir.dt.float32
    bf16 = mybir.dt.bfloat16
    AX = mybir.AxisListType.X
    COPY = mybir.ActivationFunctionType.Copy
    MULT = mybir.AluOpType.mult

    th = t_full // 2
    nblk = t_full // 128
    chk = CHK_
    nchk = th // chk
    nresc = (th - 1) // RESC

    nc = bacc.Bacc("TRN2", target_bir_lowering=False, debug=False,
                   num_devices=num_cores)
    # block-major, class-padded bf16 y_pred: [blk][ex][t-in-blk][c-pad-128]
    ypred = nc.dram_tensor("ypredb", [nblk, bn, 128, 128], bf16,
                           kind="ExternalInput")
    m2_in = nc.dram_tensor("m2", [128, SW], bf16, kind="ExternalInput")
    m2r_in = nc.dram_tensor("m2rep", [128, CHK_ * SW], bf16, kind="ExternalInput")
    mi_in = nc.dram_tensor("minit", [128, SW], bf16, kind="ExternalInput")
    oh_in = nc.dram_tensor("onehot", [C, bn * 2 * SW], bf16, kind="ExternalInput")
    h_out = nc.dram_tensor("hist", [128, max(nresc, 1)], f32, kind="ExternalOutput")
    a_out = nc.dram_tensor("afin", [128, SW + 2], bf16, kind="ExternalOutput")
    g_out = nc.dram_tensor("gfin", [128, SW], bf16, kind="ExternalOutput")

    with tile.TileContext(nc) as tc, ExitStack() as ctx:
        const = ctx.enter_context(tc.tile_pool(name="const", bufs=1))
        dramp = ctx.enter_context(tc.tile_pool(name="edram", bufs=1, space="DRAM"))
        ebf_pool = ctx.enter_context(tc.tile_pool(name="ebf", bufs=3))
        ec_pool = ctx.enter_context(tc.tile_pool(name="ec", bufs=4))
        eps_pool = ctx.enter_context(tc.tile_pool(name="eps", bufs=6, space="PSUM"))
        yts_pool = ctx.enter_context(tc.tile_pool(name="yts", bufs=2))

        m2t = const.tile([128, SW], bf16, tag="m2t")
        m2rep = const.tile([128, CHK_ * SW], bf16, tag="m2rep")
        TA = const.tile([128, 2 * WA + 2], bf16, tag="TA")
        TB = const.tile([128, 2 * WB + 2], bf16, tag="TB")
        WAt = const.tile([128, 2 * WA], bf16, tag="WAt")
        WBt = const.tile([128, 2 * WB], bf16, tag="WBt")
        alf = const.tile([128, SW + 2], bf16, tag="alf")
        hB = const.tile([128, 1], f32, tag="hB")
        mit = const.tile([128, SW], bf16, tag="mit")
        oht = const.tile([C, bn * 2 * SW], bf16, tag="oht")
        alpha = const.tile([128, SW + 2], bf16, tag="alpha")
        ut = const.tile([128, SW], bf16, tag="ut")
        vt = const.tile([128, SW], bf16, tag="vt")
        wt = const.tile([128, SW], bf16, tag="wt")
        histt = const.tile([128, max(nresc, 1)], f32, tag="histt")
        sclt = const.tile([128, 1], f32, tag="sclt")

        blk_order = []
        for i in range(nblk // 2):
            blk_order += [i, nblk - 1 - i]
        pair1, pair2 = blk_order[:2], blk_order[2:]

        nc.sync.dma_start(out=m2t[:, :], in_=m2_in.ap())
        nc.sync.dma_start(out=m2rep[:, :], in_=m2r_in.ap())
        nc.sync.dma_start(out=mit[:, :], in_=mi_in.ap())
        gw = 16 * 2 * SW

        def load_oh(g):
            nc.sync.dma_start(out=oht[:, g * gw:(g + 1) * gw],
                              in_=oh_in.ap()[:, g * gw:(g + 1) * gw])

        load_oh(0)
        load_oh(1)
        nc.vector.memset(histt[:, :], 0.0)
        nc.vector.memset(TA[:, :], 0.0)
        nc.vector.memset(TB[:, :], 0.0)
        nc.vector.memset(alf[:, :], 0.0)

        # ---- phase A helper: emissions per 128-t block via one-hot matmul ----
        edram = {}

        def emit_block(k, grp_hook=None):
            ebf_k = ebf_pool.tile([128, bn * SW], bf16)
            dirn = 0 if k < nblk // 2 else 1
            ed = dramp.tile([128, bn * SW], bf16, tag=f"ed{k}")
            edram[k] = ed
            # one yts tile per block: [c(128), ex*128t] via XBAR DMA
            # transpose straight from block-major DRAM (16-example slabs)
            ysb = yts_pool.tile([128, bn * 128], bf16)
            for grp in range(bn // 16):
                src = ypred.ap()[k, grp * 16:(grp + 1) * 16, :, :]
                nc.sync.dma_start(
                    out=ysb[:, grp * 16 * 128:(grp + 1) * 16 * 128],
                    in_=src.rearrange("e t c -> (e t) c"), transpose=True)
            for grp in range(bn // 16):
                if grp_hook is not None:
                    grp_hook(grp)
                for e in range(16):
                    ex = grp * 16 + e
                    ohoff = (ex * 2 + dirn) * SW
                    epsum = eps_pool.tile([128, SW], f32)
                    nc.tensor.matmul(
                        epsum[:, :],
                        ysb[0:C, ex * 128:(ex + 1) * 128],
                        oht[:, ohoff:ohoff + SW],
                        start=True, stop=True)
                    nc.scalar.activation(
                        ebf_k[:, ex * SW:(ex + 1) * SW], epsum[:, :],
                        COPY, bias=float(G * EPS), scale=float(G))
                # per-grp DRAM write: the last grp's write is all that
                # gates the chunk reads, not the whole block
                gsl = slice(grp * 16 * SW, (grp + 1) * 16 * SW)
                nc.gpsimd.dma_start(out=ed[:, gsl], in_=ebf_k[:, gsl])

        # ---- phase B helper: reshuffle DRAM -> (ex|dir partition, tau) ----
        ec_tiles = []
        m2rep3 = m2rep[:, :].rearrange("p (t s) -> p t s", s=SW)

        def emit_chunk_fwd(j):
            ec = ec_pool.tile([128, chk * EW], bf16)
            ec3 = ec[:, :].rearrange("p (t s) -> p t s", s=EW)
            kf = j // 4
            tl0 = (j % 4) * chk
            fsrc = edram[kf][:, :].rearrange("t (e s) -> t e s", s=SW)
            nc.sync.dma_start(
                out=ec3[0:64, :, SW:EW],
                in_=fsrc[tl0:tl0 + chk, :, :].rearrange("t e s -> e t s"))
            ec_tiles.append((ec, ec3))

        def emit_chunk_bwd(j):
            ec, ec3 = ec_tiles[j]
            kb = nblk - 1 - j // 4
            tb0 = 127 - (j % 4) * chk
            bsrc = edram[kb][:, :].rearrange("t (e s) -> t e s", s=SW)
            bslice = slice(tb0, None, -1) if tb0 - chk < 0 else slice(tb0, tb0 - chk, -1)
            nc.gpsimd.dma_start(
                out=ec3[64:128, :, SW:EW],
                in_=bsrc[bslice, :, :].rearrange("t e s -> e t s"))

        def emit_chunk_mul(j):
            ec, ec3 = ec_tiles[j]
            nc.vector.tensor_mul(ec3[:, :, 0:SW], ec3[:, :, SW:EW], m2rep3)

        # ---- phase C: two-strand interleaved DP (deps all >= dist 2) ----
        MAXO = mybir.AluOpType.max
        nr_box = [0]

        def emit_dp(tau0, tau1):
            for tau in range(tau0, tau1):
                if tau % chk == 0:
                    emit_chunk_mul(tau // chk)
                ecc = ec_tiles[tau // chk][0]
                off = (tau % chk) * EW
                ecv = ecc[:, off:off + EW].rearrange("p (h s) -> p h s", h=2)
                nc.vector.tensor_add(TA[:, WA + 2:2 * WA + 2],
                                     TA[:, 0:WA], TA[:, 1:1 + WA])
                nc.vector.tensor_add(TB[:, WB + 2:2 * WB + 2],
                                     TB[:, 0:WB], TB[:, 1:1 + WB])
                nc.vector.tensor_mul(
                    WAt[:, :].rearrange("p (h s) -> p h s", h=2),
                    TA[:, 2:2 * WA + 2].rearrange("p (h s) -> p h s", h=2),
                    ecv[:, :, 0:WA])
                nc.vector.tensor_mul(
                    WBt[:, :].rearrange("p (h s) -> p h s", h=2),
                    TB[:, 2:2 * WB + 2].rearrange("p (h s) -> p h s", h=2),
                    ecv[:, :, SA:SW])
                nc.vector.tensor_add(TA[:, 0:WA], WAt[:, 0:WA], WAt[:, WA:2 * WA])
                nc.vector.tensor_add(TB[:, 0:WB], WBt[:, 0:WB], WBt[:, WB:2 * WB])
                if tau % GREF == 0:
                    # refresh A's ghost cols from B (unscaled; rescale covers both)
                    nc.vector.tensor_copy(TA[:, SA:WA], TB[:, 0:GH])
                if tau % RESC == 0 and nr_box[0] < nresc:
                    nr = nr_box[0]
                    nc.vector.reduce_max(histt[:, nr:nr + 1], TA[:, 2:SA], axis=AX)
                    nc.vector.reduce_max(hB[:, :], TB[:, 0:WB], axis=AX)
                    nc.vector.tensor_max(histt[:, nr:nr + 1],
                                         histt[:, nr:nr + 1], hB[:, :])
                    nc.vector.reciprocal_approx_fast(sclt[:, :], histt[:, nr:nr + 1])
                    nc.vector.tensor_scalar(TB[:, 0:WB], TB[:, 0:WB],
                                            sclt[:, :], float(2.0 ** SETPOINT_LOG2),
                                            MULT, MULT)
                    nc.vector.tensor_scalar(TA[:, 0:WA], TA[:, 0:WA],
                                            sclt[:, :], float(2.0 ** SETPOINT_LOG2),
                                            MULT, MULT)
                    nr_box[0] = nr + 1

        # ---- interleaved emission: DP starts once pair-1 chunks land ----
        def _oh_hook(g):
            if g < 2:
                load_oh(g + 2)

        emit_block(pair1[0], grp_hook=_oh_hook)
        emit_block(pair1[1])
        for j in range(4):
            emit_chunk_fwd(j)
        for j in range(4):
            emit_chunk_bwd(j)

        emit_chunk_mul(0)
        ecc0 = ec_tiles[0][0]
        nc.vector.tensor_mul(TA[:, 0:WA], ecc0[:, SW:SW + WA], mit[:, 0:WA])
        nc.vector.tensor_mul(TB[:, 0:WB], ecc0[:, SW + SA:EW], mit[:, SA:SW])
        emit_dp(1, 4 * chk)

        emit_block(pair2[0])
        emit_block(pair2[1])
        for j in range(4, nchk):
            emit_chunk_fwd(j)
        for j in range(4, nchk):
            emit_chunk_bwd(j)
        emit_dp(4 * chk, th)

        # ---- final: assemble alpha, gamma on bwd rows, dump ----
        nc.vector.tensor_copy(alf[:, 0:SA], TA[:, 0:SA])
        nc.vector.tensor_copy(alf[:, SA:SW], TB[:, 0:WB])
        nc.vector.tensor_add(ut[:, :], alf[:, 0:SW], alf[:, 1:1 + SW])
        nc.vector.tensor_mul(vt[:, :], alf[:, 2:2 + SW], m2t[:, :])
        nc.vector.tensor_add(wt[:, :], ut[:, :], vt[:, :])
        nc.sync.dma_start(out=a_out.ap(), in_=alf[:, :])
        nc.sync.dma_start(out=g_out.ap(), in_=wt[:, :])
        nc.sync.dma_start(out=h_out.ap(), in_=histt[:, :])

    nc.compile()
    return nc


def kernel(y_true, y_pred):
    global _BUILT, _LAST_EXEC_NS, _LAST_RES
    from concourse.bass_utils import run_bass_kernel_spmd

    y_true = np.asarray(y_true)
    y_pred = np.ascontiguousarray(np.asarray(y_pred, dtype=np.float32))

    m2f, m2b, mif, mib, clsf, clsb = _host_metadata(y_true)

    if _BUILT is None:
        _BUILT = _build()
    nc = _BUILT

    import ml_dtypes
    bf = ml_dtypes.bfloat16
    # block-major, class-padded bf16 y_pred: [4][512 ex][128 t][128 c]
    nblk = T // 128
    ypb_all = np.zeros((B, nblk, 128, 128), bf)
    ypb_all[:, :, :, :C] = y_pred.reshape(B, nblk, 128, C).astype(bf)
    ypb_all = np.ascontiguousarray(np.swapaxes(ypb_all, 0, 1))  # [blk, B, t, c]
    in_maps = []
    for c in range(NCORES):
        sl = slice(c * BN, (c + 1) * BN)
        # mirrored layout: reverse the free (state) dim
        m2 = np.concatenate([m2f[sl], m2b[sl]], axis=0)[:, ::-1].astype(bf)
        mi = np.concatenate([mif[sl], mib[sl]], axis=0)[:, ::-1].astype(bf)
        m2rep = np.tile(m2, (1, CHK_))
        oh = np.zeros((C, BN * 2 * SW), bf)
        for e in range(BN):
            b = c * BN + e
            for dirn, cls in ((0, clsf[b]), (1, clsb[b])):
                colbase = (e * 2 + dirn) * SW
                rcls = cls[::-1]                    # mirrored
                for i in range(SW):
                    if rcls[i] >= 0:
                        oh[rcls[i], colbase + i] = bf(1.0)
        in_maps.append({
            "ypredb": np.ascontiguousarray(ypb_all[:, sl]),
            "m2": np.ascontiguousarray(m2),
            "m2rep": np.ascontiguousarray(m2rep),
            "minit": np.ascontiguousarray(mi),
            "onehot": oh,
        })

    import os
    trace = os.environ.get("CTC_TRACE", "") == "1"
    res = run_bass_kernel_spmd(nc, in_maps, list(range(NCORES)), trace=trace)
    _LAST_EXEC_NS = res.exec_time_ns
    _LAST_RES = res

    losses = np.zeros(B, np.float64)
    lng = np.log(np.float64(G))
    setlog = NRESC * SETPOINT_LOG2 * np.log(2.0)
    for c in range(NCORES):
        afin = res.results[c]["afin"].astype(np.float64)   # (128, SW+2) mirrored
        gfin = res.results[c]["gfin"].astype(np.float64)   # (128, SW) mirrored
        hist = res.results[c]["hist"].astype(np.float64)
        acc = np.log(np.maximum(hist[:, :NRESC], 1e-300)).sum(axis=1)
        afs = afin[:, 0:SW][:, ::-1]             # un-mirror -> natural state order
        gfs = gfin[:, :][:, ::-1]
        af = afs[0:64, 0:S]                      # alpha_{T/2-1}[s]
        gm = gfs[64:128, 0:S][:, ::-1]           # W_{T/2-1}[s], u -> s
        P = (af * gm).sum(axis=1)
        lnP = np.log(np.maximum(P, 1e-300))
        losses[c * BN:(c + 1) * BN] = -(
            lnP + acc[:64] + acc[64:128] - 2 * setlog - T * lng)
    return np.float32(losses.mean())



# revision 27
# speedup vs baseline: 1.1196x; 1.0129x over previous
"""CTC loss kernel for Trainium2 (8 NeuronCores, data-parallel over batch).

Strategy
--------
Per core: 64 examples. The CTC forward DP runs in probability space
with states in the free dim and (example, direction) packed into the
128 partitions: rows 0-63 run the forward DP for t=0..255, rows 64-127
run the suffix (backward) DP in state-reversed coordinates for
t=511..256.  The two halves are spliced at T/2 on host in f64:
P = sum_s alpha_255[s] * W_255[s].

Emissions E[b,t,s] = g*y_pred[b,t,ext_b[s]] are produced on the
TensorEngine: per (example, 128-t block), PE-transpose y_pred to
(class, t), then a one-hot matmul gathers all 132 state emissions for
128 t steps in one instruction; the scalar engine copies PSUM->SBUF
fusing the g scale and the f32->bf16 cast.  A DRAM round-trip
reshuffles (t-part, ex, s) into (example-part, tau-major) 32-step
chunks, landing in the ec half of an interleaved [ecm|ec] layout; a
bulk DVE multiply by the replicated skip mask fills ecm = m2*ec.

The DP is split into two independent interleaved strands so every DVE
dependency is >= 2 instructions back and same-engine semaphore waits
resolve for free (a naive schedule pays ~91ns per distance-1 wait,
3x per step): strand A owns mirrored states [0:66) plus a 16-col
ghost, strand B owns [66:132).  Mirrored state order (guards at top)
makes information flow upward-only, so B needs no ghost; A's ghost
absorbs the 2-col/step creep and is refreshed from B every 8 steps.
Each strand step is 3 ops on a packed tile T = [alpha | guard(2) | u]:
  u = alpha[0:w]+alpha[1:w+1];  W = T[2:2+2w]*[ecm|ec](strided 2-row);
  alpha' = W[0:w]+W[w:2w]
with all operands 4-byte aligned to hit the DVE 2x bf16 perf mode.

Numerics: bf16 DP state, per-32-step rescale to a 2^24 setpoint via
the fast approx reciprocal (max history written out, logs added back
on host); masked/pad one-hot columns are exactly zero (dropping the
log(y+eps) eps shifts the loss ~1e-5 relative, far under tolerance).
The final splice spans ~e^-180 for tail examples, far outside f32
range, so the final state tiles are DMA'd out and spliced on host.
"""

import numpy as np

B, T, C, L = 512, 512, 96, 64
BLANK = C - 1
EPS = 1e-7
S = 2 * L + 1          # 129 states
SW = 132               # padded state width (multiple of 4)
NCORES = 8
BN = B // NCORES       # 64 examples per core
TH = T // 2            # split point
RESC = 32              # rescale period
NRESC = (TH - 1) // RESC  # 7 rescales (tau = 32,64,...,224)
SETPOINT_LOG2 = 24     # rescale setpoint 2^24 (headroom for 32 unrescaled steps)
G = 60.646622          # exp(mean_loss/T) boost; keeps alpha ~O(1) per step
EW = 2 * SW            # per-tau DP emission width: [ecm | ec]
SA = 66                # strand A owns mirrored states [0:SA)
GREF = 8               # ghost refresh period
GH = 2 * GREF          # ghost width (16)
WA = SA + GH           # strand A computed width (82)
WB = SW - SA           # strand B width (66)

CHK_ = 32
_BUILT = None
_LAST_EXEC_NS = None
_LAST_RES = None


def _host_metadata(y_true):
    """ext labels, skip masks, init masks, per-state classes — from y_true.

    Everything is built in natural state order (validated layout), then
    reversed along the free dim at pack time for the mirrored device layout.
    """
    y_true = np.asarray(y_true, dtype=np.int32)
    lbl_len = (y_true != -1).sum(axis=-1).astype(np.int32)
    labels = np.where(y_true != -1, y_true, 0).astype(np.int32)
    ext = np.full((B, S), BLANK, np.int32)
    ext[:, 1::2] = labels
    ext_m2 = np.pad(ext[:, :-2], ((0, 0), (2, 0)), constant_values=BLANK)
    can_skip = ((ext != BLANK) & (ext != ext_m2)).astype(np.float32)

    m2f = np.zeros((B, SW), np.float32)
    m2f[:, :S] = can_skip
    m2b = np.zeros((B, SW), np.float32)
    for u in range(2, S):
        m2b[:, u] = can_skip[:, S - 1 - u + 2]

    mif = np.zeros((B, SW), np.float32)
    mif[:, 0] = 1.0
    mif[:, 1] = 1.0
    mib = np.zeros((B, SW), np.float32)
    mib[np.arange(B), S - 1 - 2 * lbl_len] = 1.0
    mib[np.arange(B), S - 1 - (2 * lbl_len - 1)] = 1.0

    clsf = np.full((B, SW), -1, np.int32)           # -1 -> all-zero column
    clsf[:, :S] = ext
    clsb = np.full((B, SW), -1, np.int32)
    clsb[:, :S] = ext[:, ::-1]
    return m2f, m2b, mif, mib, clsf, clsb


def _build(num_cores=NCORES, t_full=T, bn=BN):
    """Build and schedule the Bass module once."""
    import concourse.bacc as bacc
    import concourse.mybir as mybir
    import concourse.tile as tile
    from contextlib import ExitStack
    from concourse.vector_clock import ScopedClock

    # this walrus build allows a single sem wait per Drain: split the
    # TileContext end-drain's waits across a chain of drains.
    def _patched_drain_and_barrier(self, tick_clock, wait_clock):
        nc = self.nc
        drain_inst = nc.sync.drain()
        wait_clock.add_sem_waits(
            drain_inst.ins, ScopedClock({None: tick_clock.global_clock})
        )
        si = drain_inst.ins.sync_info
        waits = list(si.on_wait) if si and si.on_wait else []
        if len(waits) > 1:
            si.on_wait = waits[:1]
            for w in waits[1:]:
                extra = nc.sync.drain()
                esi = extra.ins.sync_info
                if esi is None:
                    extra.ins.sync_info = mybir.SyncInfo(on_wait=[w], on_update=[])
                else:
                    esi.on_wait = (esi.on_wait or []) + [w]
        nc.all_engine_barrier()
        assert self.sems is not None
        popped = nc._tile_sem_poison_stack.pop()
        assert popped is self._sem_poison
        nc.clear_and_free_semaphores(list(self.sems.allocated().values()))
        nc.all_engine_barrier()

    tile.TileContext._drain_and_barrier = _patched_drain_and_barrier

    f32 = mybir.dt.float32
    bf16 = mybir.dt.bfloat16
    AX = mybir.AxisListType.X
    COPY = mybir.ActivationFunctionType.Copy
    MULT = mybir.AluOpType.mult

    th = t_full // 2
    nblk = t_full // 128
    chk = CHK_
    nchk = th // chk
    nresc = (th - 1) // RESC

    nc = bacc.Bacc("TRN2", target_bir_lowering=False, debug=False,
                   num_devices=num_cores)
    # block-major, class-padded bf16 y_pred: [blk][ex][t-in-blk][c-pad-128]
    ypred = nc.dram_tensor("ypredb", [nblk, bn, 128, 128], bf16,
                           kind="ExternalInput")
    m2_in = nc.dram_tensor("m2", [128, SW], bf16, kind="ExternalInput")
    m2r_in = nc.dram_tensor("m2rep", [128, CHK_ * SW], bf16, kind="ExternalInput")
    mi_in = nc.dram_tensor("minit", [128, SW], bf16, kind="ExternalInput")
    oh_in = nc.dram_tensor("onehot", [C, bn * 2 * SW], bf16, kind="ExternalInput")
    h_out = nc.dram_tensor("hist", [128, max(nresc, 1)], f32, kind="ExternalOutput")
    a_out = nc.dram_tensor("afin", [128, SW + 2], bf16, kind="ExternalOutput")
    g_out = nc.dram_tensor("gfin", [128, SW], bf16, kind="ExternalOutput")

    with tile.TileContext(nc) as tc, ExitStack() as ctx:
        const = ctx.enter_context(tc.tile_pool(name="const", bufs=1))
        dramp = ctx.enter_context(tc.tile_pool(name="edram", bufs=1, space="DRAM"))
        ebf_pool = ctx.enter_context(tc.tile_pool(name="ebf", bufs=3))
        ec_pool = ctx.enter_context(tc.tile_pool(name="ec", bufs=4))
        eps_pool = ctx.enter_context(tc.tile_pool(name="eps", bufs=6, space="PSUM"))
        yts_pool = ctx.enter_context(tc.tile_pool(name="yts", bufs=2))

        m2t = const.tile([128, SW], bf16, tag="m2t")
        m2rep = const.tile([128, CHK_ * SW], bf16, tag="m2rep")
        TA = const.tile([128, 2 * WA + 2], bf16, tag="TA")
        TB = const.tile([128, 2 * WB + 2], bf16, tag="TB")
        WAt = const.tile([128, 2 * WA], bf16, tag="WAt")
        WBt = const.tile([128, 2 * WB], bf16, tag="WBt")
        alf = const.tile([128, SW + 2], bf16, tag="alf")
        hB = const.tile([128, 1], f32, tag="hB")
        mit = const.tile([128, SW], bf16, tag="mit")
        oht = const.tile([C, bn * 2 * SW], bf16, tag="oht")
        alpha = const.tile([128, SW + 2], bf16, tag="alpha")
        ut = const.tile([128, SW], bf16, tag="ut")
        vt = const.tile([128, SW], bf16, tag="vt")
        wt = const.tile([128, SW], bf16, tag="wt")
        histt = const.tile([128, max(nresc, 1)], f32, tag="histt")
        sclt = const.tile([128, 1], f32, tag="sclt")

        blk_order = []
        for i in range(nblk // 2):
            blk_order += [i, nblk - 1 - i]
        pair1, pair2 = blk_order[:2], blk_order[2:]

        nc.sync.dma_start(out=m2t[:, :], in_=m2_in.ap())
        nc.sync.dma_start(out=m2rep[:, :], in_=m2r_in.ap())
        nc.sync.dma_start(out=mit[:, :], in_=mi_in.ap())
        gw = 16 * 2 * SW

        def load_oh(g):
            nc.sync.dma_start(out=oht[:, g * gw:(g + 1) * gw],
                              in_=oh_in.ap()[:, g * gw:(g + 1) * gw])

        load_oh(0)
        load_oh(1)
        nc.vector.memset(histt[:, :], 0.0)
        nc.vector.memset(TA[:, :], 0.0)
        nc.vector.memset(TB[:, :], 0.0)
        nc.vector.memset(alf[:, :], 0.0)

        # ---- phase A helper: emissions per 128-t block via one-hot matmul ----
        edram = {}

        def emit_xbar(k):
            # one yts tile per block: [c(128), ex*128t] via XBAR DMA
            # transpose straight from block-major DRAM (16-example slabs)
            ysb = yts_pool.tile([128, bn * 128], bf16)
            for grp in range(bn // 16):
                src = ypred.ap()[k, grp * 16:(grp + 1) * 16, :, :]
                nc.sync.dma_start(
                    out=ysb[:, grp * 16 * 128:(grp + 1) * 16 * 128],
                    in_=src.rearrange("e t c -> (e t) c"), transpose=True)
            return ysb

        def emit_block(k, ysb, grp_hook=None):
            ebf_k = ebf_pool.tile([128, bn * SW], bf16)
            dirn = 0 if k < nblk // 2 else 1
            ed = dramp.tile([128, bn * SW], bf16, tag=f"ed{k}")
            edram[k] = ed
            for grp in range(bn // 16):
                if grp_hook is not None:
                    grp_hook(grp)
                for e in range(16):
                    ex = grp * 16 + e
                    ohoff = (ex * 2 + dirn) * SW
                    epsum = eps_pool.tile([128, SW], f32)
                    nc.tensor.matmul(
                        epsum[:, :],
                        ysb[0:C, ex * 128:(ex + 1) * 128],
                        oht[:, ohoff:ohoff + SW],
                        start=True, stop=True)
                    nc.scalar.activation(
                        ebf_k[:, ex * SW:(ex + 1) * SW], epsum[:, :],
                        COPY, bias=float(G * EPS), scale=float(G))
                # per-grp DRAM write: the last grp's write is all that
                # gates the chunk reads, not the whole block
                gsl = slice(grp * 16 * SW, (grp + 1) * 16 * SW)
                nc.gpsimd.dma_start(out=ed[:, gsl], in_=ebf_k[:, gsl])

        # ---- phase B helper: reshuffle DRAM -> (ex|dir partition, tau) ----
        ec_tiles = []
        m2rep3 = m2rep[:, :].rearrange("p (t s) -> p t s", s=SW)

        def emit_chunk_fwd(j):
            ec = ec_pool.tile([128, chk * EW], bf16)
            ec3 = ec[:, :].rearrange("p (t s) -> p t s", s=EW)
            kf = j // 4
            tl0 = (j % 4) * chk
            fsrc = edram[kf][:, :].rearrange("t (e s) -> t e s", s=SW)
            nc.sync.dma_start(
                out=ec3[0:64, :, SW:EW],
                in_=fsrc[tl0:tl0 + chk, :, :].rearrange("t e s -> e t s"))
            ec_tiles.append((ec, ec3))

        def emit_chunk_bwd(j, eng=None):
            ec, ec3 = ec_tiles[j]
            kb = nblk - 1 - j // 4
            tb0 = 127 - (j % 4) * chk
            bsrc = edram[kb][:, :].rearrange("t (e s) -> t e s", s=SW)
            bslice = slice(tb0, None, -1) if tb0 - chk < 0 else slice(tb0, tb0 - chk, -1)
            (eng or nc.gpsimd).dma_start(
                out=ec3[64:128, :, SW:EW],
                in_=bsrc[bslice, :, :].rearrange("t e s -> e t s"))

        def emit_chunk_mul(j):
            ec, ec3 = ec_tiles[j]
            nc.vector.tensor_mul(ec3[:, :, 0:SW], ec3[:, :, SW:EW], m2rep3)

        # ---- phase C: two-strand interleaved DP (deps all >= dist 2) ----
        MAXO = mybir.AluOpType.max
        nr_box = [0]

        def emit_dp(tau0, tau1):
            for tau in range(tau0, tau1):
                if tau % chk == 0:
                    emit_chunk_mul(tau // chk)
                ecc = ec_tiles[tau // chk][0]
                off = (tau % chk) * EW
                ecv = ecc[:, off:off + EW].rearrange("p (h s) -> p h s", h=2)
                nc.vector.tensor_add(TA[:, WA + 2:2 * WA + 2],
                                     TA[:, 0:WA], TA[:, 1:1 + WA])
                nc.vector.tensor_add(TB[:, WB + 2:2 * WB + 2],
                                     TB[:, 0:WB], TB[:, 1:1 + WB])
                nc.vector.tensor_mul(
                    WAt[:, :].rearrange("p (h s) -> p h s", h=2),
                    TA[:, 2:2 * WA + 2].rearrange("p (h s) -> p h s", h=2),
                    ecv[:, :, 0:WA])
                nc.vector.tensor_mul(
                    WBt[:, :].rearrange("p (h s) -> p h s", h=2),
                    TB[:, 2:2 * WB + 2].rearrange("p (h s) -> p h s", h=2),
                    ecv[:, :, SA:SW])
                nc.vector.tensor_add(TA[:, 0:WA], WAt[:, 0:WA], WAt[:, WA:2 * WA])
                nc.vector.tensor_add(TB[:, 0:WB], WBt[:, 0:WB], WBt[:, WB:2 * WB])
                if tau % GREF == 0:
                    # refresh A's ghost cols from B (unscaled; rescale covers both)
                    nc.vector.tensor_copy(TA[:, SA:WA], TB[:, 0:GH])
                if tau % RESC == 0 and nr_box[0] < nresc:
                    nr = nr_box[0]
                    nc.vector.reduce_max(histt[:, nr:nr + 1], TA[:, 2:SA], axis=AX)
                    nc.vector.reduce_max(hB[:, :], TB[:, 0:WB], axis=AX)
                    nc.vector.tensor_max(histt[:, nr:nr + 1],
                                         histt[:, nr:nr + 1], hB[:, :])
                    nc.vector.reciprocal_approx_fast(sclt[:, :], histt[:, nr:nr + 1])
                    nc.vector.tensor_scalar(TB[:, 0:WB], TB[:, 0:WB],
                                            sclt[:, :], float(2.0 ** SETPOINT_LOG2),
                                            MULT, MULT)
                    nc.vector.tensor_scalar(TA[:, 0:WA], TA[:, 0:WA],
                                            sclt[:, :], float(2.0 ** SETPOINT_LOG2),
                                            MULT, MULT)
                    nr_box[0] = nr + 1

        # ---- interleaved emission: DP starts once pair-1 chunks land ----
        def _oh_hook(g):
            if g < 2:
                load_oh(g + 2)

        ysb0 = emit_xbar(pair1[0])
        ysb1 = emit_xbar(pair1[1])
        emit_block(pair1[0], ysb0, grp_hook=_oh_hook)
        emit_block(pair1[1], ysb1)
        for j in range(4):
            emit_chunk_fwd(j)
        for j in range(4):
            # Act HWDGE queue: dispatched right as block-3's copies finish,
            # which is also when the ed3 write lands; all 4 fire in parallel
            emit_chunk_bwd(j, eng=nc.scalar)

        emit_chunk_mul(0)
        ecc0 = ec_tiles[0][0]
        nc.vector.tensor_mul(TA[:, 0:WA], ecc0[:, SW:SW + WA], mit[:, 0:WA])
        nc.vector.tensor_mul(TB[:, 0:WB], ecc0[:, SW + SA:EW], mit[:, SA:SW])
        emit_dp(1, 4 * chk)

        ysb2 = emit_xbar(pair2[0])
        ysb3 = emit_xbar(pair2[1])
        emit_block(pair2[0], ysb2)
        emit_block(pair2[1], ysb3)
        for j in range(4, nchk):
            emit_chunk_fwd(j)
        for j in range(4, nchk):
            emit_chunk_bwd(j)
        emit_dp(4 * chk, th)

        # ---- final: assemble alpha, gamma on bwd rows, dump ----
        nc.vector.tensor_copy(alf[:, 0:SA], TA[:, 0:SA])
        nc.vector.tensor_copy(alf[:, SA:SW], TB[:, 0:WB])
        nc.vector.tensor_add(ut[:, :], alf[:, 0:SW], alf[:, 1:1 + SW])
        nc.vector.tensor_mul(vt[:, :], alf[:, 2:2 + SW], m2t[:, :])
        nc.vector.tensor_add(wt[:, :], ut[:, :], vt[:, :])
        nc.sync.dma_start(out=a_out.ap(), in_=alf[:, :])
        nc.sync.dma_start(out=g_out.ap(), in_=wt[:, :])
        nc.sync.dma_start(out=h_out.ap(), in_=histt[:, :])

    nc.compile()
    return nc


def kernel(y_true, y_pred):
    global _BUILT, _LAST_EXEC_NS, _LAST_RES
    from concourse.bass_utils import run_bass_kernel_spmd

    y_true = np.asarray(y_true)
    y_pred = np.ascontiguousarray(np.asarray(y_pred, dtype=np.float32))

    m2f, m2b, mif, mib, clsf, clsb = _host_metadata(y_true)

    if _BUILT is None:
        _BUILT = _build()
    nc = _BUILT

    import ml_dtypes
    bf = ml_dtypes.bfloat16
    # block-major, class-padded bf16 y_pred: [4][512 ex][128 t][128 c]
    nblk = T // 128
    ypb_all = np.zeros((B, nblk, 128, 128), bf)
    ypb_all[:, :, :, :C] = y_pred.reshape(B, nblk, 128, C).astype(bf)
    ypb_all = np.ascontiguousarray(np.swapaxes(ypb_all, 0, 1))  # [blk, B, t, c]
    in_maps = []
    for c in range(NCORES):
        sl = slice(c * BN, (c + 1) * BN)
        # mirrored layout: reverse the free (state) dim
        m2 = np.concatenate([m2f[sl], m2b[sl]], axis=0)[:, ::-1].astype(bf)
        mi = np.concatenate([mif[sl], mib[sl]], axis=0)[:, ::-1].astype(bf)
        m2rep = np.tile(m2, (1, CHK_))
        oh = np.zeros((C, BN * 2 * SW), bf)
        for e in range(BN):
            b = c * BN + e
            for dirn, cls in ((0, clsf[b]), (1, clsb[b])):
                colbase = (e * 2 + dirn) * SW
                rcls = cls[::-1]                    # mirrored
                for i in range(SW):
                    if rcls[i] >= 0:
                        oh[rcls[i], colbase + i] = bf(1.0)
        in_maps.append({
            "ypredb": np.ascontiguousarray(ypb_all[:, sl]),
            "m2": np.ascontiguousarray(m2),
            "m2rep": np.ascontiguousarray(m2rep),
            "minit": np.ascontiguousarray(mi),
            "onehot": oh,
        })

    import os
    trace = os.environ.get("CTC_TRACE", "") == "1"
    res = run_bass_kernel_spmd(nc, in_maps, list(range(NCORES)), trace=trace)
    _LAST_EXEC_NS = res.exec_time_ns
    _LAST_RES = res

    losses = np.zeros(B, np.float64)
    lng = np.log(np.float64(G))
    setlog = NRESC * SETPOINT_LOG2 * np.log(2.0)
    for c in range(NCORES):
        afin = res.results[c]["afin"].astype(np.float64)   # (128, SW+2) mirrored
        gfin = res.results[c]["gfin"].astype(np.float64)   # (128, SW) mirrored
        hist = res.results[c]["hist"].astype(np.float64)
        acc = np.log(np.maximum(hist[:, :NRESC], 1e-300)).sum(axis=1)
        afs = afin[:, 0:SW][:, ::-1]             # un-mirror -> natural state order
        gfs = gfin[:, :][:, ::-1]
        af = afs[0:64, 0:S]                      # alpha_{T/2-1}[s]
        gm = gfs[64:128, 0:S][:, ::-1]           # W_{T/2-1}[s], u -> s
        P = (af * gm).sum(axis=1)
        lnP = np.log(np.maximum(P, 1e-300))
        losses[c * BN:(c + 1) * BN] = -(
            lnP + acc[:64] + acc[64:128] - 2 * setlog - T * lng)
    return np.float32(losses.mean())

